# revision 12
# baseline (speedup 1.0000x reference)
"""Trainium2 Bass kernel for nn_Attention_15908558865595.

Math: qk[b,h,s,:] is constant along the softmax axis (query is expanded
along it), and jax.nn.softmax subtracts the row max, so the attention
weights are exactly uniform (1/F). The output is therefore
    out[b,h,s,f] = mean(value[b,h,:,0])
broadcast over [S,F] — independent of query/key. The kernel computes the
per-(b,h) mean on device and broadcast-writes the 128 MiB output at the
SBUF->HBM fabric roofline. Sharding: batch*heads (32 pairs) split
4-per-core across 8 NeuronCores; no cross-device communication.

Device program per core (bh group g = 0..3, partitions k grouped 32/bh):
  vg[k,:]      = 32 value elements ++ 4 mask columns (one 18KB DMA)
  partials[k]  = sum of 32 value elements            (DVE reduce)
  masked[k,g]  = G[k,g] * partials[k],  G = (k//32==g)/F   (host const)
  bc[p,g]      = sum_k masked[k,g]  (ones.T @ masked on PE -> every
                 partition holds all 4 means)
  fill_g       = broadcast bc[:,g] to a tile         (DVE copy; bh0 uses
                 a narrow 512-col tile to launch the first DMA sooner)
  out          = one 4 MiB DMA per slab on the sync HWDGE ring; the SBUF
                 source loops the fill tile via a stride-0 middle dim.
"""
import sys

if "/opt/trn_rl_repo" not in sys.path:
    sys.path.insert(0, "/opt/trn_rl_repo")

import numpy as np

B, H, S, F = 2, 16, 1024, 1024
N_CORES = 8
BH = B * H
BH_PER_CORE = BH // N_CORES      # 4
P = 128
VCOLS = BH_PER_CORE * F // P     # 32 value elements per partition
SLAB = S * F                     # one (b,h) output slab
SLAB_COLS = SLAB // P            # 8192

_NC = None


def _g_const() -> np.ndarray:
    g = np.zeros((P, BH_PER_CORE), dtype=np.float32)
    for k in range(P):
        g[k, k // (P // BH_PER_CORE)] = 1.0 / F
    return g


def _build():
    import concourse.bacc as bacc
    import concourse.bass as bass
    import concourse.tile as tile
    from concourse import mybir

    nc = bacc.Bacc("TRN2", target_bir_lowering=False, debug=False, num_devices=N_CORES)

    vg_ap = nc.dram_tensor(
        "vg", [P, VCOLS + BH_PER_CORE], mybir.dt.float32, kind="ExternalInput"
    ).ap()
    out_ap = nc.dram_tensor(
        "out", [BH_PER_CORE * SLAB], mybir.dt.float32, kind="ExternalOutput"
    ).ap()

    with tile.TileContext(nc) as tc:
        with tc.tile_pool(name="small", bufs=1) as small, \
             tc.tile_pool(name="psum", bufs=1, space="PSUM") as psum, \
             tc.tile_pool(name="fills", bufs=1) as fills:
            vgtile = small.tile([P, VCOLS + BH_PER_CORE], mybir.dt.float32)
            nc.scalar.dma_start(vgtile[:], vg_ap[:])

            ones = small.tile([P, P], mybir.dt.float32)
            nc.vector.memset(ones[:], 1.0)

            partials = small.tile([P, 1], mybir.dt.float32)
            nc.vector.reduce_sum(
                partials[:], vgtile[:, 0:VCOLS], axis=mybir.AxisListType.X
            )

            masked = small.tile([P, BH_PER_CORE], mybir.dt.float32)
            nc.vector.tensor_scalar_mul(
                masked[:], vgtile[:, VCOLS : VCOLS + BH_PER_CORE], partials[:, 0:1]
            )

            bc_psum = psum.tile([P, BH_PER_CORE], mybir.dt.float32)
            nc.tensor.matmul(bc_psum[:], ones[:], masked[:], start=True, stop=True)
            bc = small.tile([P, BH_PER_CORE], mybir.dt.float32)
            nc.vector.tensor_copy(out=bc[:], in_=bc_psum[:])

            # Slab 0 leads with a narrow tile so the first output DMA can
            # launch as soon as bc lands; everything else streams from wide
            # tiles (16 KB per-partition descriptors for best DMA efficiency).
            # Per-slab plan: (start_col, tile_cols, reps) covering 8192 cols.
            plans = [
                [(0, 512, 1), (512, 3840, 2)],
                [(0, 4096, 2)],
                [(0, 4096, 2)],
                [(0, 4096, 2)],
            ]
            for i, plan in enumerate(plans):
                for start, cols, reps in plan:
                    fill = fills.tile(
                        [P, cols], mybir.dt.float32, tag=f"fill{i}_{start}"
                    )
                    nc.vector.tensor_copy(
                        out=fill[:], in_=bc[:, i : i + 1].to_broadcast((P, cols))
                    )
                    # One DMA covers reps*cols columns of the slab; the SBUF
                    # source loops the fill tile via a stride-0 middle dim.
                    dst = out_ap[bass.ts(i, SLAB)].rearrange(
                        "(p y) -> p y", p=P
                    )[:, start : start + reps * cols].rearrange(
                        "p (r x) -> p r x", x=cols
                    )
                    src = fill[:, None, :].to_broadcast((P, reps, cols))
                    nc.sync.dma_start(dst, src)
    nc.compile()
    return nc


def _get_nc():
    global _NC
    if _NC is None:
        _NC = _build()
    return _NC


def run_device(value_flat: np.ndarray, **spmd_kwargs):
    """value_flat: [BH, F] f32. Returns (out [BH, S, F], BassKernelResults)."""
    from concourse.bass_utils import run_bass_kernel_spmd

    nc = _get_nc()
    g = _g_const()
    in_maps = [
        {
            "vg": np.ascontiguousarray(
                np.concatenate(
                    [
                        value_flat[c * BH_PER_CORE : (c + 1) * BH_PER_CORE].reshape(
                            P, VCOLS
                        ),
                        g,
                    ],
                    axis=1,
                )
            )
        }
        for c in range(N_CORES)
    ]
    res = run_bass_kernel_spmd(nc, in_maps, list(range(N_CORES)), **spmd_kwargs)
    out = np.empty((BH, S, F), dtype=np.float32)
    for c in range(N_CORES):
        out[c * BH_PER_CORE : (c + 1) * BH_PER_CORE] = res.results[c]["out"].reshape(
            BH_PER_CORE, S, F
        )
    return out, res


def kernel(query: np.ndarray, key: np.ndarray, value: np.ndarray) -> np.ndarray:
    value_flat = np.ascontiguousarray(
        np.asarray(value, dtype=np.float32).reshape(BH, F)
    )
    out, _ = run_device(value_flat)
    return out.reshape(B, H, S, F)


# revision 13
# speedup vs baseline: 1.1463x; 1.1463x over previous
"""Trainium2 Bass kernel for nn_Attention_15908558865595.

Math: qk[b,h,s,:] is constant along the softmax axis (query is expanded
along it), and jax.nn.softmax subtracts the row max, so the attention
weights are exactly uniform (1/F). The output is therefore
    out[b,h,s,f] = mean(value[b,h,:,0])
broadcast over [S,F] — independent of query/key. The kernel computes the
per-(b,h) mean on device and broadcast-writes the 128 MiB output at the
SBUF->HBM fabric roofline. Sharding: batch*heads (32 pairs) split
4-per-core across 8 NeuronCores; no cross-device communication.

Device program per core (bh group g = 0..3, partitions k grouped 32/bh):
  vg[k,:]      = 32 value elements ++ 4 mask columns (one 18KB DMA)
  partials[k]  = sum of 32 value elements            (DVE reduce)
  masked[k,g]  = G[k,g] * partials[k],  G = (k//32==g)/F   (host const)
  bc[p,g]      = sum_k masked[k,g]  (ones.T @ masked on PE -> every
                 partition holds all 4 means)
  fill_g       = broadcast bc[:,g] to a tile         (DVE copy; bh0 uses
                 a narrow 512-col tile to launch the first DMA sooner)
  out          = one 4 MiB DMA per slab on the sync HWDGE ring; the SBUF
                 source loops the fill tile via a stride-0 middle dim.
"""
import sys

if "/opt/trn_rl_repo" not in sys.path:
    sys.path.insert(0, "/opt/trn_rl_repo")

import numpy as np

B, H, S, F = 2, 16, 1024, 1024
N_CORES = 8
BH = B * H
BH_PER_CORE = BH // N_CORES      # 4
P = 128
VCOLS = BH_PER_CORE * F // P     # 32 value elements per partition
SLAB = S * F                     # one (b,h) output slab
SLAB_COLS = SLAB // P            # 8192

_NC = None


def _g_const() -> np.ndarray:
    g = np.zeros((P, BH_PER_CORE), dtype=np.float32)
    for k in range(P):
        g[k, k // (P // BH_PER_CORE)] = 1.0 / F
    return g


def _build():
    import concourse.bacc as bacc
    import concourse.bass as bass
    import concourse.tile as tile
    from concourse import mybir

    nc = bacc.Bacc("TRN2", target_bir_lowering=False, debug=False, num_devices=N_CORES)

    vg_ap = nc.dram_tensor(
        "vg", [P, VCOLS + BH_PER_CORE], mybir.dt.float32, kind="ExternalInput"
    ).ap()
    out_ap = nc.dram_tensor(
        "out", [BH_PER_CORE * SLAB], mybir.dt.float32, kind="ExternalOutput"
    ).ap()

    with tile.TileContext(nc) as tc:
        with tc.tile_pool(name="small", bufs=1) as small, \
             tc.tile_pool(name="psum", bufs=1, space="PSUM") as psum, \
             tc.tile_pool(name="fills", bufs=1) as fills:
            vgtile = small.tile([P, VCOLS + BH_PER_CORE], mybir.dt.float32)
            nc.scalar.dma_start(vgtile[:], vg_ap[:])

            ones = small.tile([P, P], mybir.dt.float32)
            nc.vector.memset(ones[:], 1.0)

            partials = small.tile([P, 1], mybir.dt.float32)
            nc.vector.reduce_sum(
                partials[:], vgtile[:, 0:VCOLS], axis=mybir.AxisListType.X
            )

            masked = small.tile([P, BH_PER_CORE], mybir.dt.float32)
            nc.vector.tensor_scalar_mul(
                masked[:], vgtile[:, VCOLS : VCOLS + BH_PER_CORE], partials[:, 0:1]
            )

            bc_psum = psum.tile([P, BH_PER_CORE], mybir.dt.float32)
            nc.tensor.matmul(bc_psum[:], ones[:], masked[:], start=True, stop=True)
            bc = small.tile([P, BH_PER_CORE], mybir.dt.float32)
            nc.vector.tensor_copy(out=bc[:], in_=bc_psum[:])

            # Slab 0 leads with a narrow tile so the first output DMA can
            # launch as soon as bc lands; everything else streams from wide
            # tiles (16 KB per-partition descriptors for best DMA efficiency).
            # Per-slab plan: (start_col, tile_cols, reps) covering 8192 cols.
            plans = [
                [(0, 1024, 8)],
                [(0, 1024, 8)],
                [(0, 1024, 8)],
                [(0, 1024, 8)],
            ]
            for i, plan in enumerate(plans):
                for start, cols, reps in plan:
                    fill = fills.tile(
                        [P, cols], mybir.dt.float32, tag=f"fill{i}_{start}"
                    )
                    nc.vector.tensor_copy(
                        out=fill[:], in_=bc[:, i : i + 1].to_broadcast((P, cols))
                    )
                    # One DMA covers reps*cols columns of the slab; the SBUF
                    # source loops the fill tile via a stride-0 middle dim.
                    dst = out_ap[bass.ts(i, SLAB)].rearrange(
                        "(p y) -> p y", p=P
                    )[:, start : start + reps * cols].rearrange(
                        "p (r x) -> p r x", x=cols
                    )
                    src = fill[:, None, :].to_broadcast((P, reps, cols))
                    nc.sync.dma_start(dst, src)
    nc.compile()
    return nc


def _get_nc():
    global _NC
    if _NC is None:
        _NC = _build()
    return _NC


def run_device(value_flat: np.ndarray, **spmd_kwargs):
    """value_flat: [BH, F] f32. Returns (out [BH, S, F], BassKernelResults)."""
    from concourse.bass_utils import run_bass_kernel_spmd

    nc = _get_nc()
    g = _g_const()
    in_maps = [
        {
            "vg": np.ascontiguousarray(
                np.concatenate(
                    [
                        value_flat[c * BH_PER_CORE : (c + 1) * BH_PER_CORE].reshape(
                            P, VCOLS
                        ),
                        g,
                    ],
                    axis=1,
                )
            )
        }
        for c in range(N_CORES)
    ]
    res = run_bass_kernel_spmd(nc, in_maps, list(range(N_CORES)), **spmd_kwargs)
    out = np.empty((BH, S, F), dtype=np.float32)
    for c in range(N_CORES):
        out[c * BH_PER_CORE : (c + 1) * BH_PER_CORE] = res.results[c]["out"].reshape(
            BH_PER_CORE, S, F
        )
    return out, res


def kernel(query: np.ndarray, key: np.ndarray, value: np.ndarray) -> np.ndarray:
    value_flat = np.ascontiguousarray(
        np.asarray(value, dtype=np.float32).reshape(BH, F)
    )
    out, _ = run_device(value_flat)
    return out.reshape(B, H, S, F)
